# revision 1
# baseline (speedup 1.0000x reference)
"""XNOR-Net BasicBlock forward (BN-sign-binconv-PReLU x2 + BN + residual + PReLU)
distributed over 8 Trainium2 NeuronCores, data-parallel over the batch axis.

Self-contained: hardcodes shapes N=64, C=128, H=W=56, 8 cores.
"""

import numpy as np
import ml_dtypes

import concourse.bass as bass
import concourse.mybir as mybir
import concourse.tile as tile
from concourse import bacc
from concourse.bass_utils import run_bass_kernel_spmd

F32 = mybir.dt.float32
F16 = mybir.dt.float16
BF16 = mybir.dt.bfloat16
FP8 = mybir.dt.float8e4
PITCH = 64
AF = mybir.ActivationFunctionType
OP = mybir.AluOpType

N_CORES = 8
N_LOC = 8          # images per core
C = 128            # channels (== partitions)
H = W = 56
HW = H * W         # 3136
PADW = W + 2       # 58
EPS = 1e-5
TILE_ROWS = 7      # output rows per PSUM tile -> 7*64-span = 448 <= 512 (one bank)
N_TILES = H // TILE_ROWS   # 8 -> 4 uniform groups of 2, 4-deep PSUM rotation
CHUNK = TILE_ROWS * W      # 392
N_CHUNKS = HW // CHUNK     # 8

# pp param columns
P_S1, P_S2, P_G1, P_B1, P_G2, P_B2, P_G3, P_B3, P_A1, P_A2, P_A3 = range(11)
NP = 11


def _col(t, j):
    return t[:, j : j + 1]


def _rstd_from_allreduced(nc, pool, ar, name):
    """ar: [128,2] = sum over cores of [mean_i, var_i + mean_i^2].
    Returns (mean, rstd) tiles [128,1] f32 with rstd = 1/sqrt(var+EPS),
    Newton-refined to cover ScalarE Sqrt spline error."""
    mean = pool.tile([C, 1], F32, name=f"mean_{name}", tag=f"mean_{name}")
    ex2 = pool.tile([C, 1], F32, name=f"ex2_{name}", tag="sc_ex2")
    nc.vector.tensor_scalar_mul(mean[:], _col(ar, 0), 1.0 / N_CORES)
    nc.vector.tensor_scalar_mul(ex2[:], _col(ar, 1), 1.0 / N_CORES)
    negmean = pool.tile([C, 1], F32, name=f"negmean_{name}", tag="sc_negmean")
    nc.vector.tensor_scalar_mul(negmean[:], mean[:], -1.0)
    vpe = pool.tile([C, 1], F32, name=f"vpe_{name}", tag="sc_vpe")
    # vpe = ex2 - mean^2 + EPS  == (mean * -mean) add ex2, then +EPS
    nc.vector.scalar_tensor_tensor(vpe[:], mean[:], negmean[:], ex2[:], OP.mult, OP.add)
    nc.vector.tensor_scalar_add(vpe[:], vpe[:], EPS)
    rec = pool.tile([C, 1], F32, name=f"rec_{name}", tag="sc_rec")
    nc.vector.reciprocal(rec[:], vpe[:])
    rstd = pool.tile([C, 1], F32, name=f"rstd_{name}", tag=f"rstd_{name}")
    nc.scalar.activation(rstd[:], rec[:], AF.Sqrt)
    # Newton: y <- y * (1.5 - 0.5 * vpe * y^2)
    t1 = pool.tile([C, 1], F32, name=f"t1_{name}", tag="sc_t1")
    nc.vector.tensor_tensor(out=t1[:], in0=rstd[:], in1=rstd[:], op=OP.mult)
    nc.vector.tensor_tensor(out=t1[:], in0=t1[:], in1=vpe[:], op=OP.mult)
    nc.vector.tensor_scalar(t1[:], t1[:], -0.5, 1.5, OP.mult, OP.add)
    nc.vector.tensor_tensor(out=rstd[:], in0=rstd[:], in1=t1[:], op=OP.mult)
    return mean, rstd


def _affine_consts(nc, pool, pp, mean, rstd, g_col, b_col, name):
    """k = g * rstd ; cb = b - mean * k. Returns (k, cb) tiles [128,1]."""
    k = pool.tile([C, 1], F32, name=f"k_{name}", tag=f"k_{name}")
    nc.vector.tensor_tensor(out=k[:], in0=_col(pp, g_col), in1=rstd[:], op=OP.mult)
    negk = pool.tile([C, 1], F32, name=f"negk_{name}", tag="sc_negk")
    nc.vector.tensor_scalar_mul(negk[:], k[:], -1.0)
    cb = pool.tile([C, 1], F32, name=f"cb_{name}", tag=f"cb_{name}")
    nc.vector.scalar_tensor_tensor(
        cb[:], mean[:], negk[:], _col(pp, b_col), OP.mult, OP.add
    )
    return k, cb


def _sign_threshold(nc, pool, k, cb, ra, rs, name):
    """b = sign(k*prelu(s*c) + cb) == Sign(c*sgn - sgn*tau) for monotone prelu
    (a>0). ra=1/a, rs=1/s precomputed. Returns (sgn, nbias) [128,1] tiles."""
    negcb = pool.tile([C, 1], F32, name=f"negcb_{name}", tag="sc_negcb")
    nc.vector.tensor_scalar_mul(negcb[:], cb[:], -1.0)
    rk = pool.tile([C, 1], F32, name=f"rk_{name}", tag="sc_rk")
    nc.vector.reciprocal(rk[:], k[:])
    t2 = pool.tile([C, 1], F32, name=f"t2_{name}", tag="sc_t2")
    nc.vector.tensor_tensor(out=t2[:], in0=negcb[:], in1=rk[:], op=OP.mult)
    # prelu^-1(t2) = max(t2,0) + min(t2,0)/a
    tpos = pool.tile([C, 1], F32, name=f"tpos_{name}", tag="sc_tpos")
    nc.vector.tensor_scalar_max(tpos[:], t2[:], 0.0)
    tneg = pool.tile([C, 1], F32, name=f"tneg_{name}", tag="sc_tneg")
    nc.vector.tensor_scalar_min(tneg[:], t2[:], 0.0)
    pinv = pool.tile([C, 1], F32, name=f"pinv_{name}", tag="sc_pinv")
    nc.vector.scalar_tensor_tensor(pinv[:], tneg[:], ra[:], tpos[:],
                                   OP.mult, OP.add)
    tau = pool.tile([C, 1], F32, name=f"tau_{name}", tag="sc_tau")
    nc.vector.tensor_tensor(out=tau[:], in0=pinv[:], in1=rs[:], op=OP.mult)
    sgn = pool.tile([C, 1], F32, name=f"sgn_{name}", tag=f"sgn_{name}")
    nc.scalar.activation(sgn[:], k[:], AF.Sign)
    nbias = pool.tile([C, 1], F32, name=f"nbias_{name}", tag=f"nbias_{name}")
    nc.vector.tensor_tensor(out=nbias[:], in0=sgn[:], in1=tau[:], op=OP.mult)
    nc.vector.tensor_scalar_mul(nbias[:], nbias[:], -1.0)
    return sgn, nbias


def build_nc(dbg=False, reps=1):
    nc = bacc.Bacc(None, target_bir_lowering=False, debug=False, num_devices=N_CORES)

    x_d = nc.dram_tensor("x", [N_LOC, C, HW], F32, kind="ExternalInput")
    xb_d = nc.dram_tensor("xb", [N_LOC, C, HW], F16, kind="ExternalInput")
    w1_d = nc.dram_tensor("w1t", [9, C, C], FP8, kind="ExternalInput")
    w2_d = nc.dram_tensor("w2t", [9, C, C], FP8, kind="ExternalInput")
    pp_d = nc.dram_tensor("pp", [C, NP], F32, kind="ExternalInput")
    out_d = nc.dram_tensor("out", [N_LOC, C, HW], F16, kind="ExternalOutput")
    if dbg:
        dbg_pad_d = nc.dram_tensor("dbg_pad", [C, H + 2, PITCH], FP8,
                                   kind="ExternalOutput")
        dbg_c1_d = nc.dram_tensor("dbg_c1", [C, N_LOC, HW], F16,
                                  kind="ExternalOutput")
        dbg_c2_d = nc.dram_tensor("dbg_c2", [C, N_LOC, HW], F16,
                                  kind="ExternalOutput")
        dbg_k_d = nc.dram_tensor("dbg_k", [C, 10], F32, kind="ExternalOutput")

    with tile.TileContext(nc) as tc:
        with (
            tc.tile_pool(name="const", bufs=1) as const,
            tc.tile_pool(name="work", bufs=2) as work,
            tc.tile_pool(name="psum", bufs=2, space="PSUM") as psum,
            tc.tile_pool(name="dram", bufs=1, space="DRAM") as dram,
        ):
            # ---- persistent SBUF tensors ----
            pp = const.tile([C, NP], F32)
            nc.gpsimd.dma_start(pp[:], pp_d[:])
            w1s = const.tile([C, 9, C], FP8)
            w2s = const.tile([C, 9, C], FP8)
            for t in range(9):
                nc.gpsimd.dma_start(w1s[:, t, :], w1_d[t])
                nc.gpsimd.dma_start(w2s[:, t, :], w2_d[t])
            c1f = const.tile([C, N_LOC, HW], F16)   # conv1 raw integer outputs
            c2f = const.tile([C, N_LOC, HW], F16)   # conv2 raw integer outputs
            stats1 = const.tile([C, N_LOC * N_CHUNKS, 6], F32, tag="stats")
            stats2 = const.tile([C, N_LOC * N_CHUNKS, 6], F32, tag="stats")
            stats3 = const.tile([C, N_LOC * N_CHUNKS, 6], F32, tag="stats")
            pads = []
            for j in range(2):
                # +1 spare zero row: tile-6 dh=2 taps read 2 elements past
                # row 57 for garbage output columns (skipped at evacuation)
                p = const.tile([C, H + 3, PITCH], FP8, name=f"pad{j}")
                nc.vector.memset(p[:], 0.0)
                pads.append(p)

            a1 = _col(pp, P_A1)
            a2 = _col(pp, P_A2)
            a3 = _col(pp, P_A3)
            s1 = _col(pp, P_S1)
            s2 = _col(pp, P_S2)

            ra1 = const.tile([C, 1], F32, name="ra1")
            nc.vector.reciprocal(ra1[:], a1)
            rs1 = const.tile([C, 1], F32, name="rs1")
            nc.vector.reciprocal(rs1[:], s1)

            cc_counter = [0]

            def reduce_stats(stats, idx):
                """bn_aggr + pack [mean, var+mean^2] + allreduce; returns [128,2] tile."""
                mv = const.tile([C, 2], F32, name=f"mv{idx}", tag="sc_mv")
                nc.vector.bn_aggr(mv[:], stats[:])
                e = const.tile([C, 2], F32, name=f"e{idx}", tag="sc_e")
                nc.vector.tensor_copy(_col(e, 0), _col(mv, 0))
                nc.vector.scalar_tensor_tensor(
                    _col(e, 1), _col(mv, 0), _col(mv, 0), _col(mv, 1), OP.mult, OP.add
                )
                n = cc_counter[0]
                cc_counter[0] += 1
                cci = dram.tile([C, 2], F32, name=f"cc_in{n}", tag=f"cc_in{n}")
                cco = dram.tile([N_CORES, C, 2], F32, name=f"cc_out{n}",
                                tag=f"cc_out{n}", addr_space="Shared")
                nc.sync.dma_start(cci[:], e[:])
                nc.gpsimd.collective_compute(
                    "AllGather",
                    OP.bypass,
                    replica_groups=[list(range(N_CORES))],
                    ins=[cci.opt()],
                    outs=[cco.opt()],
                )
                g8 = const.tile([C, 2, N_CORES], F32, name=f"g8{idx}", tag="sc_g8")
                for r in range(N_CORES):
                    nc.sync.dma_start(g8[:, :, r], cco[r])
                g = const.tile([C, 2], F32, name=f"g{idx}", tag="sc_g")
                nc.vector.tensor_reduce(g[:], g8[:], mybir.AxisListType.X, OP.add)
                return g

            QSPAN = TILE_ROWS * PITCH  # 512: flat padded span per tile

            def conv(pad, ws, dst, stats_to=None):
                """3x3 conv of padded +/-1 fp8 image (row pitch 64) with 9 [C,C]
                taps -> dst [C,HW] f16. Vertical tap pairs (dh=0,1) run as fp8
                DoubleRow matmuls (256-deep contraction); dh=2 taps run as
                plain fp8 matmuls. Outputs computed over the flat padded span
                (8 garbage cols per row skipped at evacuation)."""
                padf = pad[:].rearrange("p r w -> p (r w)")
                wbase = ws[:, 0, :]
                for g0 in range(0, N_TILES, 2):
                    tiles = range(g0, min(g0 + 2, N_TILES))
                    ng = len(tiles)
                    # one PSUM tile spanning the group's banks (512 f32 = 1 bank each)
                    # each sub-tile padded to a full 512-elem bank so the
                    # matmul target never crosses a bank boundary
                    psg = psum.tile([C, ng, 512], F32,
                                    tag="ps",
                                    name=f"psg{g0 // 2}", bufs=3)
                    for dw in range(3):
                        # pair lhsT: taps (0,dw) and (1,dw); tap stride = 3*C
                        wp = bass.AP(wbase.tensor, wbase.offset + dw * C,
                                     [list(wbase.ap[0]), [3 * C, 2], [1, C]])
                        for j, t in enumerate(tiles):
                            q0 = t * QSPAN + dw
                            rhs = bass.AP(padf.tensor, padf.offset + q0,
                                          [list(padf.ap[0]), [PITCH, 2],
                                           [1, QSPAN]])
                            nc.tensor.matmul(
                                psg[:, j, 0:QSPAN], wp, rhs, start=(dw == 0),
                                stop=False,
                                perf_mode=mybir.MatmulPerfMode.DoubleRow,
                            )
                    for dw in range(3):
                        for j, t in enumerate(tiles):
                            q0 = t * QSPAN + 2 * PITCH + dw
                            nc.tensor.matmul(
                                psg[:, j, 0:QSPAN], ws[:, 6 + dw, :],
                                padf[:, q0 : q0 + QSPAN],
                                start=False, stop=(dw == 2),
                            )
                    # single strided evacuation for the whole group;
                    # first group of each image goes via ScalarE (DVE relief)
                    gbase = psg[:]
                    src_ap = bass.AP(gbase.tensor, gbase.offset,
                                     [list(gbase.ap[0]), [512, ng],
                                      [PITCH, TILE_ROWS], [1, W]])
                    dst_ap = dst[:, g0 * CHUNK : (g0 + ng) * CHUNK].rearrange(
                        "p (g r w) -> p g r w", r=TILE_ROWS, w=W)
                    nc.vector.tensor_copy(dst_ap, src_ap)
                    if stats_to is not None:
                        stats, i, sc, al = stats_to
                        pst = work.tile([C, 2 * CHUNK], F32, tag="pstat",
                                        bufs=2)
                        nc.scalar.activation(
                            pst[:, 0 : ng * CHUNK],
                            dst[:, g0 * CHUNK : (g0 + ng) * CHUNK],
                            AF.Prelu, scale=sc, alpha=al)
                        for j, t in enumerate(tiles):
                            nc.vector.bn_stats(
                                stats[:, i * N_CHUNKS + t, :],
                                pst[:, j * CHUNK : (j + 1) * CHUNK])

            def image_stats(src, stats, i):
                for cch in range(N_CHUNKS):
                    nc.vector.bn_stats(
                        stats[:, i * N_CHUNKS + cch, :],
                        src[:, cch * CHUNK : (cch + 1) * CHUNK],
                    )

            for _rep in range(reps):
                # ================= Phase A: BN1 stats =================
                for i in range(N_LOC):
                    xin = work.tile([C, HW], F32, tag="xin", bufs=3)
                    q = HW // 4
                    for qq in range(4):
                        nc.sync.dma_start(xin[:, qq * q : (qq + 1) * q],
                                          x_d[i, :, qq * q : (qq + 1) * q])
                    image_stats(xin, stats1, i)

                g1ar = reduce_stats(stats1, 0)
                mean1, rstd1 = _rstd_from_allreduced(nc, const, g1ar, "1")
                k1, c1b = _affine_consts(nc, const, pp, mean1, rstd1, P_G1, P_B1, "1")

                # ================= Phase B: b1 = sign(BN1(x)); conv1; stats2 ========
                for i in range(N_LOC):
                    xin = work.tile([C, HW], F32, tag="xin", bufs=3)
                    qb = HW // 4
                    for qq in range(4):
                        nc.sync.dma_start(xin[:, qq * qb : (qq + 1) * qb],
                                          x_d[i, :, qq * qb : (qq + 1) * qb])
                    pad = pads[i % 2]
                    nc.scalar.activation(
                        pad[:, 1 : H + 1, 1 : W + 1],
                        xin.rearrange("p (h w) -> p h w", h=H, w=W),
                        AF.Sign,
                        bias=c1b[:],
                        scale=k1[:],
                    )
                    if dbg and i == 0:
                        nc.sync.dma_start(dbg_pad_d[:], pad[:])
                    conv(pad, w1s, c1f[:, i, :], stats_to=(stats2, i, s1[:], a1[:]))

                g2ar = reduce_stats(stats2, 1)
                mean2, rstd2 = _rstd_from_allreduced(nc, const, g2ar, "2")
                k2, c2b = _affine_consts(nc, const, pp, mean2, rstd2, P_G2, P_B2, "2")

                # ================= Phase C: b2 = sign(BN2(p1)); conv2; stats3 =======
                sgn2, nbias2 = _sign_threshold(nc, const, k2, c2b, ra1[:], rs1[:], f"2")
                for i in range(N_LOC):
                    pad = pads[i % 2]
                    nc.scalar.activation(
                        pad[:, 1 : H + 1, 1 : W + 1],
                        c1f[:, i, :].rearrange("p (h w) -> p h w", h=H, w=W),
                        AF.Sign,
                        bias=nbias2[:],
                        scale=sgn2[:],
                    )
                    conv(pad, w2s, c2f[:, i, :], stats_to=(stats3, i, s2[:], a2[:]))

                g3ar = reduce_stats(stats3, 2)
                mean3, rstd3 = _rstd_from_allreduced(nc, const, g3ar, "3")
                k3, c3b = _affine_consts(nc, const, pp, mean3, rstd3, P_G3, P_B3, "3")

                if dbg:
                    nc.sync.dma_start(dbg_c1_d[:], c1f[:])
                    nc.sync.dma_start(dbg_c2_d[:], c2f[:])
                    dbgk = const.tile([C, 10], F32)
                    for j, t_ in enumerate(
                        [k1, c1b, k2, c2b, k3, c3b, mean1, rstd1, mean2, rstd2]
                    ):
                        nc.vector.tensor_copy(_col(dbgk, j), t_[:])
                    nc.sync.dma_start(dbg_k_d[:], dbgk[:])

                # ====== Phase D: y = PReLU(BN3(PReLU(s2*c2)) + x) ======
                for i in range(N_LOC):
                    xbt = work.tile([C, HW], F16, tag="xbt", bufs=2)
                    nc.sync.dma_start(xbt[:], xb_d[i])
                    p2t = work.tile([C, HW], F16, tag="f32a", bufs=2)
                    nc.scalar.activation(
                        p2t[:], c2f[:, i, :], AF.Prelu, scale=s2[:], alpha=a2[:]
                    )
                    wv = work.tile([C, HW], F32, tag="f32b", bufs=2)
                    nc.vector.scalar_tensor_tensor(
                        wv[:], p2t[:], k3[:], xbt[:], OP.mult, OP.add
                    )
                    yout = work.tile([C, HW], F16, tag="xin", bufs=3)
                    nc.scalar.activation(
                        yout[:], wv[:], AF.Prelu, bias=c3b[:], alpha=a3[:]
                    )
                    nc.sync.dma_start(out_d[i], yout[:])

    nc.compile()
    return nc


def _prep_host(x, bn1_g, bn1_b, w1, prelu1_a, bn2_g, bn2_b, w2, prelu2_a,
               bn3_g, bn3_b, prelu3_a):
    def wprep(w_flat):
        w = np.asarray(w_flat, np.float32).reshape(C, C, 3, 3)
        scale = np.mean(np.abs(w), axis=(1, 2, 3)).astype(np.float32)  # [C]
        # lhsT layout [tap, i, o] = sign(w[o, i, dh, dw])
        wT = np.sign(w).transpose(2, 3, 1, 0).reshape(9, C, C)
        return wT.astype(mybir.dt.np(FP8)), scale

    w1t, s1 = wprep(w1)
    w2t, s2 = wprep(w2)

    pp = np.zeros((C, NP), np.float32)
    pp[:, P_S1] = s1
    pp[:, P_S2] = s2
    pp[:, P_G1] = np.asarray(bn1_g, np.float32)
    pp[:, P_B1] = np.asarray(bn1_b, np.float32)
    pp[:, P_G2] = np.asarray(bn2_g, np.float32)
    pp[:, P_B2] = np.asarray(bn2_b, np.float32)
    pp[:, P_G3] = np.asarray(bn3_g, np.float32)
    pp[:, P_B3] = np.asarray(bn3_b, np.float32)
    pp[:, P_A1] = np.float32(prelu1_a)
    pp[:, P_A2] = np.float32(prelu2_a)
    pp[:, P_A3] = np.float32(prelu3_a)

    x = np.ascontiguousarray(np.asarray(x, np.float32).reshape(64, C, HW))
    xb = x.astype(np.float16)
    in_maps = []
    for r in range(N_CORES):
        in_maps.append({
            "x": x[r * N_LOC : (r + 1) * N_LOC],
            "xb": xb[r * N_LOC : (r + 1) * N_LOC],
            "w1t": w1t,
            "w2t": w2t,
            "pp": pp,
        })
    return in_maps


_NC_CACHE = None


def _get_nc():
    global _NC_CACHE
    if _NC_CACHE is None:
        _NC_CACHE = build_nc()
    return _NC_CACHE


def run(in_maps, **kwargs):
    nc = _get_nc()
    return run_bass_kernel_spmd(nc, in_maps, core_ids=list(range(N_CORES)), **kwargs)


def kernel(**inputs):
    in_maps = _prep_host(**inputs)
    last_err = None
    for attempt in range(3):
        try:
            res = run(in_maps)
            break
        except Exception as e:  # transient NRT device errors happen; retry
            last_err = e
            import time as _time
            _time.sleep(2.0)
    else:
        raise last_err
    out = np.concatenate(
        [np.asarray(r["out"]).astype(np.float32).reshape(N_LOC, C, H, W)
         for r in res.results], axis=0
    )
    return out


if __name__ == "__main__":
    rng = np.random.default_rng(0)
    x = rng.standard_normal((64, C, H, W), dtype=np.float32)
    w1 = ((rng.random((C * C * 9, 1), dtype=np.float32) - 0.5) * 0.002)
    w2 = ((rng.random((C * C * 9, 1), dtype=np.float32) - 0.5) * 0.002)
    ones = np.ones(C, np.float32)
    zeros = np.zeros(C, np.float32)
    y = kernel(x=x, bn1_g=ones, bn1_b=zeros, w1=w1, prelu1_a=np.float32(0.25),
               bn2_g=ones, bn2_b=zeros, w2=w2, prelu2_a=np.float32(0.25),
               bn3_g=ones, bn3_b=zeros, prelu3_a=np.float32(0.25))
    print("out", y.shape, y.dtype, float(np.abs(y).mean()))



# revision 35
# speedup vs baseline: 1.2302x; 1.2302x over previous
"""XNOR-Net BasicBlock forward (BN-sign-binconv-PReLU x2 + BN + residual + PReLU)
distributed over 8 Trainium2 NeuronCores, data-parallel over the batch axis.

Self-contained: hardcodes shapes N=64, C=128, H=W=56, 8 cores.

Structure (per core, 8 images):
  A: load x, bn_stats -> allreduce -> BN1 consts
  B: reload x, sign1 (ACT) -> conv1 (5 fp8 DoubleRow passes/tile) -> evac raw
     c1 (f16 ints, ACT/DVE split) + prelu pst (ACT/DVE split) + bn_stats
     -> allreduce -> BN2 consts
  C: sign2 from c1 via threshold trick (ACT; DVE path for some images)
     -> conv2 -> fused prelu evac p2=prelu(s2*c2) f16 (ACT) + bn_stats
     xb prefetch into c1's dead slots -> allreduce -> BN3 consts
  D: p2k = k3*p2+cb3 (DVE ts), vb = p2k+xb (DVE tt), yout = prelu (ACT), store
"""

import os
import numpy as np
import ml_dtypes

XK_CONV5 = os.environ.get("XK_CONV5", "1") == "1"
XK_HOIST = os.environ.get("XK_HOIST", "1") == "1"
XK_DVESIGN = os.environ.get("XK_DVESIGN", "1") == "1"
XK_DSPLIT = os.environ.get("XK_DSPLIT", "1") == "1"

import concourse.bass as bass
import concourse.mybir as mybir
import concourse.tile as tile
from concourse import bacc
from concourse.bass_utils import run_bass_kernel_spmd

F32 = mybir.dt.float32
F16 = mybir.dt.float16
FP8 = mybir.dt.float8e4
PITCH = 64
AF = mybir.ActivationFunctionType
OP = mybir.AluOpType

N_CORES = 8
N_LOC = 8          # images per core
C = 128            # channels (== partitions)
H = W = 56
HW = H * W         # 3136
PADW = W + 2       # 58
EPS = 1e-5
TILE_ROWS = 7      # output rows per PSUM tile -> 7*64-span = 448 <= 512
N_TILES = H // TILE_ROWS   # 8
CHUNK = TILE_ROWS * W      # 392
QSPAN = TILE_ROWS * PITCH  # 448
GCHUNK = 2 * CHUNK         # 784 per evac group
N_GROUPS = 4               # evac groups per image (2 tiles each)
ACHUNK = 392               # phase-A bn_stats chunk (2 per quarter-load)
N_ACH = HW // ACHUNK       # 8
N_DVE_SIGN = 3             # phase-C images whose sign runs on DVE

# pp param columns
P_S1, P_S2, P_G1, P_B1, P_G2, P_B2, P_G3, P_B3, P_A1, P_A2, P_A3 = range(11)
NP = 11


def _col(t, j):
    return t[:, j : j + 1]


def _rstd_from_allreduced(nc, pool, ar, name):
    """ar: [128,2] = sum over cores of [mean_i, var_i + mean_i^2].
    Returns (mean, rstd) tiles [128,1] f32 with rstd = 1/sqrt(var+EPS),
    Newton-refined to cover ScalarE Sqrt spline error."""
    mean = pool.tile([C, 1], F32, name=f"mean_{name}", tag=f"mean_{name}")
    ex2 = pool.tile([C, 1], F32, name=f"ex2_{name}", tag="sc_ex2")
    nc.vector.tensor_scalar_mul(mean[:], _col(ar, 0), 1.0 / N_CORES)
    nc.vector.tensor_scalar_mul(ex2[:], _col(ar, 1), 1.0 / N_CORES)
    negmean = pool.tile([C, 1], F32, name=f"negmean_{name}", tag="sc_negmean")
    nc.vector.tensor_scalar_mul(negmean[:], mean[:], -1.0)
    vpe = pool.tile([C, 1], F32, name=f"vpe_{name}", tag="sc_vpe")
    nc.vector.scalar_tensor_tensor(vpe[:], mean[:], negmean[:], ex2[:], OP.mult, OP.add)
    nc.vector.tensor_scalar_add(vpe[:], vpe[:], EPS)
    rec = pool.tile([C, 1], F32, name=f"rec_{name}", tag="sc_rec")
    nc.vector.reciprocal(rec[:], vpe[:])
    rstd = pool.tile([C, 1], F32, name=f"rstd_{name}", tag=f"rstd_{name}")
    nc.scalar.activation(rstd[:], rec[:], AF.Sqrt)
    t1 = pool.tile([C, 1], F32, name=f"t1_{name}", tag="sc_t1")
    nc.vector.tensor_tensor(out=t1[:], in0=rstd[:], in1=rstd[:], op=OP.mult)
    nc.vector.tensor_tensor(out=t1[:], in0=t1[:], in1=vpe[:], op=OP.mult)
    nc.vector.tensor_scalar(t1[:], t1[:], -0.5, 1.5, OP.mult, OP.add)
    nc.vector.tensor_tensor(out=rstd[:], in0=rstd[:], in1=t1[:], op=OP.mult)
    return mean, rstd


def _affine_consts(nc, pool, pp, mean, rstd, g_col, b_col, name):
    """k = g * rstd ; cb = b - mean * k. Returns (k, cb) tiles [128,1]."""
    k = pool.tile([C, 1], F32, name=f"k_{name}", tag=f"k_{name}")
    nc.vector.tensor_tensor(out=k[:], in0=_col(pp, g_col), in1=rstd[:], op=OP.mult)
    negk = pool.tile([C, 1], F32, name=f"negk_{name}", tag="sc_negk")
    nc.vector.tensor_scalar_mul(negk[:], k[:], -1.0)
    cb = pool.tile([C, 1], F32, name=f"cb_{name}", tag=f"cb_{name}")
    nc.vector.scalar_tensor_tensor(
        cb[:], mean[:], negk[:], _col(pp, b_col), OP.mult, OP.add
    )
    return k, cb


def _sign_threshold(nc, pool, k, cb, ra, rs, name):
    """b = sign(k*prelu(s*c) + cb) == sgn * sign(c - tau) for monotone prelu
    (a>0). ra=1/a, rs=1/s precomputed.
    Returns (sgn, nbias, tau, twos, negs) [128,1] tiles:
      ACT path: Sign(sgn*c + nbias), nbias = -sgn*tau
      DVE path: b01 = (c >= tau); pad = b01*twos + negs, twos=2*sgn, negs=-sgn
    """
    negcb = pool.tile([C, 1], F32, name=f"negcb_{name}", tag="sc_negcb")
    nc.vector.tensor_scalar_mul(negcb[:], cb[:], -1.0)
    rk = pool.tile([C, 1], F32, name=f"rk_{name}", tag="sc_rk")
    nc.vector.reciprocal(rk[:], k[:])
    t2 = pool.tile([C, 1], F32, name=f"t2_{name}", tag="sc_t2")
    nc.vector.tensor_tensor(out=t2[:], in0=negcb[:], in1=rk[:], op=OP.mult)
    tpos = pool.tile([C, 1], F32, name=f"tpos_{name}", tag="sc_tpos")
    nc.vector.tensor_scalar_max(tpos[:], t2[:], 0.0)
    tneg = pool.tile([C, 1], F32, name=f"tneg_{name}", tag="sc_tneg")
    nc.vector.tensor_scalar_min(tneg[:], t2[:], 0.0)
    pinv = pool.tile([C, 1], F32, name=f"pinv_{name}", tag="sc_pinv")
    nc.vector.scalar_tensor_tensor(pinv[:], tneg[:], ra[:], tpos[:],
                                   OP.mult, OP.add)
    tau = pool.tile([C, 1], F32, name=f"tau_{name}", tag=f"tau_{name}")
    nc.vector.tensor_tensor(out=tau[:], in0=pinv[:], in1=rs[:], op=OP.mult)
    sgn = pool.tile([C, 1], F32, name=f"sgn_{name}", tag=f"sgn_{name}")
    nc.scalar.activation(sgn[:], k[:], AF.Sign)
    nbias = pool.tile([C, 1], F32, name=f"nbias_{name}", tag=f"nbias_{name}")
    nc.vector.tensor_tensor(out=nbias[:], in0=sgn[:], in1=tau[:], op=OP.mult)
    nc.vector.tensor_scalar_mul(nbias[:], nbias[:], -1.0)
    twos = pool.tile([C, 1], F32, name=f"twos_{name}", tag=f"twos_{name}")
    nc.vector.tensor_scalar_mul(twos[:], sgn[:], 2.0)
    negs = pool.tile([C, 1], F32, name=f"negs_{name}", tag=f"negs_{name}")
    nc.vector.tensor_scalar_mul(negs[:], sgn[:], -1.0)
    return sgn, nbias, tau, twos, negs


def build_nc(dbg=False):
    nc = bacc.Bacc(None, target_bir_lowering=False, debug=False, num_devices=N_CORES)

    x_d = nc.dram_tensor("x", [N_LOC, C, HW], F32, kind="ExternalInput")
    xb_d = nc.dram_tensor("xb", [N_LOC, C, HW], F16, kind="ExternalInput")
    w1_d = nc.dram_tensor("w1t", [10, C, C], FP8, kind="ExternalInput")
    w2_d = nc.dram_tensor("w2t", [10, C, C], FP8, kind="ExternalInput")
    pp_d = nc.dram_tensor("pp", [C, NP], F32, kind="ExternalInput")
    out_d = nc.dram_tensor("out", [N_LOC, C, HW], F16, kind="ExternalOutput")

    with tile.TileContext(nc) as tc:
        with (
            tc.tile_pool(name="const", bufs=1) as const,
            tc.tile_pool(name="work", bufs=2) as work,
            tc.tile_pool(name="psum", bufs=2, space="PSUM") as psum,
            tc.tile_pool(name="dram", bufs=1, space="DRAM") as dram,
        ):
            # ---- persistent SBUF tensors ----
            pp = const.tile([C, NP], F32)
            nc.gpsimd.dma_start(pp[:], pp_d[:])
            w1s = const.tile([C, 10, C], FP8)
            w2s = const.tile([C, 10, C], FP8)
            for t in range(10):
                nc.gpsimd.dma_start(w1s[:, t, :], w1_d[t])
                nc.gpsimd.dma_start(w2s[:, t, :], w2_d[t])
            c1f = const.tile([C, N_LOC, HW], F16)   # conv1 raw ints; xb later
            p2f = const.tile([C, N_LOC, HW], F16)   # prelu(s2*conv2)
            stats1 = const.tile([C, N_LOC * N_ACH, 6], F32, tag="stats")
            stats2 = const.tile([C, N_LOC * N_GROUPS * 2, 6], F32, tag="stats")
            stats3 = const.tile([C, N_LOC * N_GROUPS * 2, 6], F32, tag="stats")
            pads = []
            for j in range(2):
                # +1 spare zero row: tile-7 dh=2 taps read past row 57 for
                # garbage output columns (skipped at evacuation)
                p = const.tile([C, H + 3, PITCH], FP8, name=f"pad{j}")
                nc.vector.memset(p[:], 0.0)
                pads.append(p)

            a1 = _col(pp, P_A1)
            a2 = _col(pp, P_A2)
            a3 = _col(pp, P_A3)
            s1 = _col(pp, P_S1)
            s2 = _col(pp, P_S2)

            ra1 = const.tile([C, 1], F32, name="ra1")
            nc.vector.reciprocal(ra1[:], a1)
            rs1 = const.tile([C, 1], F32, name="rs1")
            nc.vector.reciprocal(rs1[:], s1)
            # s1a = s1*a1 ; s1o = s1*(1-a1)  (for DVE prelu of raw c1)
            s1a = const.tile([C, 1], F32, name="s1a")
            nc.vector.tensor_tensor(out=s1a[:], in0=s1, in1=a1, op=OP.mult)
            s1o = const.tile([C, 1], F32, name="s1o")
            nc.vector.scalar_tensor_tensor(s1o[:], s1a[:], -1.0, s1,
                                           OP.mult, OP.add)

            cc_counter = [0]

            def reduce_stats(stats, idx):
                """bn_aggr + pack [mean, var+mean^2] + allgather-sum."""
                mv = const.tile([C, 2], F32, name=f"mv{idx}", tag="sc_mv")
                nc.vector.bn_aggr(mv[:], stats[:])
                e = const.tile([C, 2], F32, name=f"e{idx}", tag="sc_e")
                nc.vector.tensor_copy(_col(e, 0), _col(mv, 0))
                nc.vector.scalar_tensor_tensor(
                    _col(e, 1), _col(mv, 0), _col(mv, 0), _col(mv, 1), OP.mult, OP.add
                )
                n = cc_counter[0]
                cc_counter[0] += 1
                cci = dram.tile([C, 2], F32, name=f"cc_in{n}", tag=f"cc_in{n}")
                cco = dram.tile([N_CORES, C, 2], F32, name=f"cc_out{n}",
                                tag=f"cc_out{n}", addr_space="Shared")
                nc.sync.dma_start(cci[:], e[:])
                nc.gpsimd.collective_compute(
                    "AllGather",
                    OP.bypass,
                    replica_groups=[list(range(N_CORES))],
                    ins=[cci.opt()],
                    outs=[cco.opt()],
                )
                g8 = const.tile([C, 2, N_CORES], F32, name=f"g8{idx}", tag="sc_g8")
                nc.scalar.dma_start(g8[:], cco[:].rearrange("r c j -> c j r"))
                g = const.tile([C, 2], F32, name=f"g{idx}", tag="sc_g")
                nc.vector.tensor_reduce(g[:], g8[:], mybir.AxisListType.X, OP.add)
                return g

            def conv(pad, ws, groups_cb, mid_cb=None):
                """3x3 conv of padded +/-1 fp8 image (row pitch 64) with taps
                as 5 fp8 DoubleRow passes (vertical pairs dh0+dh1 per dw;
                horizontal pairs (dh2,dw0)+(dh2,dw1) and (dh2,dw2)+zero-tap).
                groups_cb(g0, psg, tiles) evacuates each 2-tile group.
                mid_cb (if set) is issued after group 0's matmuls, before its
                evacuation — used to hoist the next image's sign into the
                ACT queue so ACT never idles waiting for matmuls."""
                padf = pad[:].rearrange("p r w -> p (r w)")
                wbase = ws[:, 0, :]
                for g0 in range(0, N_TILES, 2):
                    tiles = range(g0, g0 + 2)
                    psg = psum.tile([C, 2, 512], F32, tag="ps",
                                    name=f"psg{g0 // 2}", bufs=3)
                    for dw in range(3):
                        wp = bass.AP(wbase.tensor, wbase.offset + dw * C,
                                     [list(wbase.ap[0]), [3 * C, 2], [1, C]])
                        for j, t in enumerate(tiles):
                            q0 = t * QSPAN + dw
                            rhs = bass.AP(padf.tensor, padf.offset + q0,
                                          [list(padf.ap[0]), [PITCH, 2],
                                           [1, QSPAN]])
                            nc.tensor.matmul(
                                psg[:, j, 0:QSPAN], wp, rhs, start=(dw == 0),
                                stop=False,
                                perf_mode=mybir.MatmulPerfMode.DoubleRow,
                            )
                    if XK_CONV5:
                        # dh=2 taps paired with the zero tap (index 9):
                        # weight pair stride (9-tap)*C is a multiple of 16;
                        # rhs pair stride 0 re-reads the same span (nullified
                        # by the zero weights). Stride-1 horizontal pairs
                        # violate the DoubleRow step%16 ISA rule and crash.
                        for pi, tap in enumerate((6, 7, 8)):
                            wp = bass.AP(wbase.tensor, wbase.offset + tap * C,
                                         [list(wbase.ap[0]),
                                          [(9 - tap) * C, 2], [1, C]])
                            for j, t in enumerate(tiles):
                                q0 = t * QSPAN + 2 * PITCH + (tap - 6)
                                rhs = bass.AP(padf.tensor, padf.offset + q0,
                                              [list(padf.ap[0]), [0, 2],
                                               [1, QSPAN]])
                                nc.tensor.matmul(
                                    psg[:, j, 0:QSPAN], wp, rhs, start=False,
                                    stop=(pi == 2),
                                    perf_mode=mybir.MatmulPerfMode.DoubleRow,
                                )
                    else:
                        for dw in range(3):
                            for j, t in enumerate(tiles):
                                q0 = t * QSPAN + 2 * PITCH + dw
                                nc.tensor.matmul(
                                    psg[:, j, 0:QSPAN], ws[:, 6 + dw, :],
                                    padf[:, q0 : q0 + QSPAN],
                                    start=False, stop=(dw == 2),
                                )
                    if g0 == 0 and mid_cb is not None:
                        mid_cb()
                    groups_cb(g0 // 2, psg)

            def psum_src(psg):
                """strided AP over a 2-tile PSUM group skipping pitch garbage:
                [C, 2, TILE_ROWS, W]"""
                gbase = psg[:]
                return bass.AP(gbase.tensor, gbase.offset,
                               [list(gbase.ap[0]), [512, 2],
                                [PITCH, TILE_ROWS], [1, W]])

            # ================= Phase A: BN1 stats =================
            # half-image tiles: finer DMA/stats rotation removes load stalls
            XH = HW // 2  # 1568 = rows 28h..28h+27

            def load_xh(i, h):
                xh = work.tile([C, XH], F32, tag="xin", bufs=5, name="xin")
                QTR = XH // 2  # 784
                for qq in range(2):
                    nc.sync.dma_start(
                        xh[:, qq * QTR : (qq + 1) * QTR],
                        x_d[i, :, h * XH + qq * QTR : h * XH + (qq + 1) * QTR])
                return xh

            for i in range(N_LOC):
                for h in range(2):
                    xh = load_xh(i, h)
                    for ch in range(4):
                        nc.vector.bn_stats(
                            stats1[:, i * N_ACH + h * 4 + ch, :],
                            xh[:, ch * ACHUNK : (ch + 1) * ACHUNK])

            g1ar = reduce_stats(stats1, 0)
            mean1, rstd1 = _rstd_from_allreduced(nc, const, g1ar, "1")
            k1, c1b = _affine_consts(nc, const, pp, mean1, rstd1, P_G1, P_B1, "1")

            # ========== Phase B: b1 = sign(BN1(x)); conv1; c1 + stats2 ======
            def load_x(i):
                return (load_xh(i, 0), load_xh(i, 1))

            def sign1(i, xpair):
                for h in range(2):
                    nc.scalar.activation(
                        pads[i % 2][:, 1 + 28 * h : 1 + 28 * (h + 1), 1 : W + 1],
                        xpair[h].rearrange("p (h w) -> p h w", h=28, w=W),
                        AF.Sign,
                        bias=c1b[:],
                        scale=k1[:],
                    )

            xin_cur = load_x(0)
            sign1(0, xin_cur)
            for i in range(N_LOC):
                xin_next = load_x(i + 1) if i + 1 < N_LOC else None

                def mid_b(i=i, xn=xin_next):
                    if XK_HOIST and xn is not None:
                        sign1(i + 1, xn)

                if not XK_HOIST and i > 0:
                    sign1(i, xin_cur)

                last = i == N_LOC - 1

                def evac_b(g, psg, i=i, last=last):
                    dst = c1f[:, i, g * GCHUNK : (g + 1) * GCHUNK].rearrange(
                        "p (g r w) -> p g r w", r=TILE_ROWS, w=W)
                    src = psum_src(psg)
                    cg = c1f[:, i, g * GCHUNK : (g + 1) * GCHUNK]
                    if g < 3 or last:   # ACT raw evac
                        nc.scalar.activation(dst, src, AF.Copy)
                    else:       # DVE raw evac
                        nc.vector.tensor_copy(dst, src)
                    if g < 2 or last:   # ACT prelu -> pst
                        pst = work.tile([C, GCHUNK], F16, tag="pst", bufs=2)
                        nc.scalar.activation(pst[:], cg, AF.Prelu,
                                             scale=s1[:], alpha=a1[:])
                    else:       # DVE prelu -> pst (ts+ts+tt)
                        ya = work.tile([C, GCHUNK], F16, tag="ya", bufs=1)
                        nc.vector.tensor_scalar(ya[:], cg, s1a[:], None,
                                                OP.mult)
                        m2p = work.tile([C, GCHUNK], F16, tag="m2p", bufs=1)
                        nc.vector.tensor_scalar(m2p[:], cg, 0.0, s1o[:],
                                                OP.max, OP.mult)
                        pst = work.tile([C, GCHUNK], F16, tag="pst", bufs=2)
                        nc.vector.tensor_tensor(out=pst[:], in0=ya[:],
                                                in1=m2p[:], op=OP.add)
                    for h in range(2):
                        nc.vector.bn_stats(
                            stats2[:, (i * N_GROUPS + g) * 2 + h, :],
                            pst[:, h * CHUNK : (h + 1) * CHUNK])

                conv(pads[i % 2], w1s, evac_b, mid_cb=mid_b)
                xin_cur = xin_next

            g2ar = reduce_stats(stats2, 1)
            mean2, rstd2 = _rstd_from_allreduced(nc, const, g2ar, "2")
            k2, c2b = _affine_consts(nc, const, pp, mean2, rstd2, P_G2, P_B2, "2")
            sgn2, nbias2, tau2, twos2, negs2 = _sign_threshold(
                nc, const, k2, c2b, ra1[:], rs1[:], "2")

            # ===== Phase C: b2 = sgn2*sign(c1-tau2); conv2; p2 + stats3 =====
            def sign2(i, split=False):
                ci = c1f[:, i, :]
                if XK_DVESIGN and i % 2 == 1:  # DVE path (ACT relief)
                    b01 = work.tile([C, HW], F16, tag="b01", bufs=1,
                                    name="b01")
                    nc.vector.tensor_scalar(b01[:], ci, tau2[:], None, OP.is_ge)
                    nc.vector.tensor_scalar(
                        pads[i % 2][:, 1 : H + 1, 1 : W + 1],
                        b01.rearrange("p (h w) -> p h w", h=H, w=W),
                        twos2[:], negs2[:], OP.mult, OP.add)
                else:
                    cv = ci.rearrange("p (h w) -> p h w", h=H, w=W)
                    halves = ((0, 28), (28, 56)) if split else ((0, 56),)
                    for h0, h1 in halves:
                        nc.scalar.activation(
                            pads[i % 2][:, 1 + h0 : 1 + h1, 1 : W + 1],
                            cv[:, h0:h1, :],
                            AF.Sign,
                            bias=nbias2[:],
                            scale=sgn2[:],
                        )

            sign2(0, split=True)
            for i in range(N_LOC):

                def mid_c(i=i):
                    if XK_HOIST and i + 1 < N_LOC:
                        sign2(i + 1)

                if not XK_HOIST and i > 0:
                    sign2(i)

                def evac_c(g, psg, i=i):
                    dst = p2f[:, i, g * GCHUNK : (g + 1) * GCHUNK].rearrange(
                        "p (g r w) -> p g r w", r=TILE_ROWS, w=W)
                    nc.scalar.activation(dst, psum_src(psg), AF.Prelu,
                                         scale=s2[:], alpha=a2[:])
                    pg = p2f[:, i, g * GCHUNK : (g + 1) * GCHUNK]
                    for h in range(2):
                        nc.vector.bn_stats(
                            stats3[:, (i * N_GROUPS + g) * 2 + h, :],
                            pg[:, h * CHUNK : (h + 1) * CHUNK])

                conv(pads[i % 2], w2s, evac_c, mid_cb=mid_c)
                # prefetch residual into c1's dead slot (same dtype/shape)
                nc.sync.dma_start(c1f[:, i, :], xb_d[i])

            g3ar = reduce_stats(stats3, 2)
            mean3, rstd3 = _rstd_from_allreduced(nc, const, g3ar, "3")
            k3, c3b = _affine_consts(nc, const, pp, mean3, rstd3, P_G3, P_B3, "3")

            # ====== Phase D: y = PReLU(k3*p2 + c3b + x) ======
            oma3 = const.tile([C, 1], F32, name="oma3")
            nc.vector.tensor_scalar(oma3[:], a3, -1.0, 1.0, OP.mult, OP.add)
            HH = HW // 2
            for i in range(N_LOC):
                p2k = work.tile([C, HW], F16, tag="p2k", bufs=2)
                vb = work.tile([C, HW], F16, tag="vb", bufs=2)
                yout = work.tile([C, HW], F16, tag="yout", bufs=4)
                # first/last image run in halves to shorten fill and tail
                spans = (((0, HH), (HH, HW))
                         if (XK_DSPLIT and i in (0, N_LOC - 1))
                         else ((0, HW),))
                for s0, s1_ in spans:
                    nc.vector.tensor_scalar(p2k[:, s0:s1_], p2f[:, i, s0:s1_],
                                            k3[:], c3b[:], OP.mult, OP.add)
                    nc.vector.tensor_tensor(out=vb[:, s0:s1_],
                                            in0=p2k[:, s0:s1_],
                                            in1=c1f[:, i, s0:s1_], op=OP.add)
                    nc.scalar.activation(yout[:, s0:s1_], vb[:, s0:s1_],
                                         AF.Prelu, alpha=a3[:])
                if XK_DSPLIT and i == N_LOC - 1:  # split last store
                    nc.sync.dma_start(out_d[i, :, 0:HH], yout[:, 0:HH])
                    nc.scalar.dma_start(out_d[i, :, HH:], yout[:, HH:])
                else:
                    nc.sync.dma_start(out_d[i], yout[:])

    nc.compile()
    return nc


def _prep_host(x, bn1_g, bn1_b, w1, prelu1_a, bn2_g, bn2_b, w2, prelu2_a,
               bn3_g, bn3_b, prelu3_a):
    def wprep(w_flat):
        w = np.asarray(w_flat, np.float32).reshape(C, C, 3, 3)
        scale = np.mean(np.abs(w), axis=(1, 2, 3)).astype(np.float32)  # [C]
        # lhsT layout [tap, i, o] = sign(w[o, i, dh, dw]); tap 9 = zeros
        wT = np.zeros((10, C, C), np.float32)
        wT[:9] = np.sign(w).transpose(2, 3, 1, 0).reshape(9, C, C)
        return wT.astype(mybir.dt.np(FP8)), scale

    w1t, s1 = wprep(w1)
    w2t, s2 = wprep(w2)

    pp = np.zeros((C, NP), np.float32)
    pp[:, P_S1] = s1
    pp[:, P_S2] = s2
    pp[:, P_G1] = np.asarray(bn1_g, np.float32)
    pp[:, P_B1] = np.asarray(bn1_b, np.float32)
    pp[:, P_G2] = np.asarray(bn2_g, np.float32)
    pp[:, P_B2] = np.asarray(bn2_b, np.float32)
    pp[:, P_G3] = np.asarray(bn3_g, np.float32)
    pp[:, P_B3] = np.asarray(bn3_b, np.float32)
    pp[:, P_A1] = np.float32(prelu1_a)
    pp[:, P_A2] = np.float32(prelu2_a)
    pp[:, P_A3] = np.float32(prelu3_a)

    x = np.ascontiguousarray(np.asarray(x, np.float32).reshape(64, C, HW))
    xb = x.astype(np.float16)
    in_maps = []
    for r in range(N_CORES):
        in_maps.append({
            "x": x[r * N_LOC : (r + 1) * N_LOC],
            "xb": xb[r * N_LOC : (r + 1) * N_LOC],
            "w1t": w1t,
            "w2t": w2t,
            "pp": pp,
        })
    return in_maps


_NC_CACHE = None


def _get_nc():
    global _NC_CACHE
    if _NC_CACHE is None:
        _NC_CACHE = build_nc()
    return _NC_CACHE


def run(in_maps, **kwargs):
    nc = _get_nc()
    return run_bass_kernel_spmd(nc, in_maps, core_ids=list(range(N_CORES)), **kwargs)


def kernel(**inputs):
    in_maps = _prep_host(**inputs)
    last_err = None
    for attempt in range(3):
        try:
            res = run(in_maps)
            break
        except Exception as e:  # transient NRT device errors happen; retry
            last_err = e
            import time as _time
            _time.sleep(2.0)
    else:
        raise last_err
    out = np.concatenate(
        [np.asarray(r["out"]).astype(np.float32).reshape(N_LOC, C, H, W)
         for r in res.results], axis=0
    )
    return out


if __name__ == "__main__":
    rng = np.random.default_rng(0)
    x = rng.standard_normal((64, C, H, W), dtype=np.float32)
    w1 = ((rng.random((C * C * 9, 1), dtype=np.float32) - 0.5) * 0.002)
    w2 = ((rng.random((C * C * 9, 1), dtype=np.float32) - 0.5) * 0.002)
    ones = np.ones(C, np.float32)
    zeros = np.zeros(C, np.float32)
    y = kernel(x=x, bn1_g=ones, bn1_b=zeros, w1=w1, prelu1_a=np.float32(0.25),
               bn2_g=ones, bn2_b=zeros, w2=w2, prelu2_a=np.float32(0.25),
               bn3_g=ones, bn3_b=zeros, prelu3_a=np.float32(0.25))
    print("out", y.shape, y.dtype, float(np.abs(y).mean()))
